# revision 19
# baseline (speedup 1.0000x reference)
"""SAGAN-style self-attention on 8 TRN2 NeuronCores, pure data-parallel.

Reference computation (per batch element, CH=64, H=W=64, N=4096, M=1024):
    theta = W_theta @ x          [8, N]
    phi   = pool(W_phi @ x)      [8, M]
    g     = pool(W_g @ x)        [32, M]
    E     = exp(theta^T phi)     [N, M]
    o     = W_o @ (g @ beta^T),  beta = E / rowsum(E)
    out   = gamma * o + x

Per-core strategy (2 batch elements), v2:
  - conv weight matrix [64, 128] emits theta/phi REPLICATED at partition
    bases {0, 32, 96} and g at 64-95, so score matmuls (K=8) pack 3-way
    into distinct PE row-groups via tile_position and run concurrently
    (~3x PE throughput on the output-bound scores phase).
  - 2x2 maxpool over ALL 128 partitions in one DVE pass (cost is
    free-dim-bound, so pooling the replicas costs nothing extra).
  - W_o folded into g up front (G2T pair layout for fp8 DoubleRow av
    matmul, ones-column emits the softmax denominator for free).
  - exp split across ScalarE (table Exp -> fp8) and DVE (fast-exp:
    tensor_scalar affine -> int8 bitcast as fp8e4m3; same quality class
    as fp8 quantization of exact exp).
  - normalize: denominator DMA-reshaped for 64-lane reciprocal, gamma
    folded into the reciprocal, gpsimd partition_broadcast; residual
    out = t + x on gpsimd with f32 x (bit-exact passthrough at gamma=0).
"""

import os
import sys

import numpy as np

if "/opt/trn_rl_repo" not in sys.path:
    sys.path.insert(0, "/opt/trn_rl_repo")

import ml_dtypes

B, CH, H, W = 16, 64, 64, 64
N = H * W          # 4096 queries
M = N // 4         # 1024 keys (after 2x2 pool)
NCORES = 8
BPC = B // NCORES  # 2 batch elements per core

NB = 1024          # normalize block width
NNB = N // NB      # 4 per batch element
NCH = 512          # attention n-chunk width
NCHUNK = N // NCH  # 8 chunks per batch element

# fast-exp constants: fp8e4m3 bits u = round(A*s + BEXP) reinterpreted as
# fp8 approximate exp(s).  A = 8/ln2; BEXP = 8*7 (bias) - 0.45 (mantissa
# PWL centering, Schraudolph-style).
FE_A = float(8.0 / np.log(2.0))
FE_B = 56.0 - 0.45

_BUILT = None


def _build():
    global _BUILT
    if _BUILT is not None:
        return _BUILT

    from contextlib import ExitStack

    import concourse.bass as bass
    import concourse.mybir as mybir
    import concourse.tile as tile
    from concourse import bacc

    f32 = mybir.dt.float32
    bf16 = mybir.dt.bfloat16
    fp8 = mybir.dt.float8e4
    i8 = mybir.dt.int8
    DR = mybir.MatmulPerfMode.DoubleRow
    ts = bass.ts
    Exp = mybir.ActivationFunctionType.Exp
    Copy = mybir.ActivationFunctionType.Copy
    amax = mybir.AluOpType.max
    amult = mybir.AluOpType.mult
    aadd = mybir.AluOpType.add

    nc = bacc.Bacc("TRN2", target_bir_lowering=False, debug=False)

    x_d = nc.dram_tensor("x", [BPC, 64, N], f32, kind="ExternalInput")
    xbf_d = nc.dram_tensor("xbf", [BPC, 64, N], bf16, kind="ExternalInput")
    w2_d = nc.dram_tensor("w2", [64, 128], bf16, kind="ExternalInput")
    wot_d = nc.dram_tensor("wot", [32, 64], bf16, kind="ExternalInput")
    gcol_d = nc.dram_tensor("gcol", [64, 1], f32, kind="ExternalInput")
    out_d = nc.dram_tensor("out", [BPC, 64, N], f32, kind="ExternalOutput")
    scr_d = nc.dram_tensor("scr", [16, NCH], bf16, kind="Internal")

    # theta replica bases for 3-way packed score matmuls
    RB = (0, 32, 96)

    with tile.TileContext(nc) as tc, ExitStack() as ctx:
        pool = lambda name, bufs, **kw: ctx.enter_context(
            tc.tile_pool(name=name, bufs=bufs, **kw)
        )
        const_p = pool("const", 1)
        xb_p = pool("xb", 1)
        thp_p = pool("thp", 1)
        pgp_p = pool("pg", 1)
        g2t_p = pool("g2t", 1)
        e_p = pool("esb", 8)
        rec_p = pool("rec", 2)
        rb_p = pool("rb", 2)
        tt_p = pool("tt", 2)
        ou_p = pool("ou", 3)

        # ---- load constants + inputs (xbf first: conv-critical) -----------
        xbf = []
        for b in range(BPC):
            xbf.append(xb_p.tile([64, N], bf16, tag=f"xbf{b}", name=f"xbf{b}"))
        for cc in range(2):
            nc.sync.dma_start(xbf[0][:, ts(cc, 512)], xbf_d[0, :, ts(cc, 512)])
        w2_sb = const_p.tile([64, 128], bf16, tag="w2", name="w2")
        nc.sync.dma_start(w2_sb[:], w2_d[:, :])
        wot_sb = const_p.tile([128, 64], bf16, tag="wot", name="wot")
        nc.sync.dma_start(wot_sb[64:96, :], wot_d[:, :])
        gcol_sb = const_p.tile([64, 1], f32, tag="gcol", name="gcol")
        nc.sync.dma_start(gcol_sb[:], gcol_d[:, :])
        for cc in range(2, 8):
            nc.sync.dma_start(xbf[0][:, ts(cc, 512)], xbf_d[0, :, ts(cc, 512)])
        for cc in range(8):
            nc.sync.dma_start(xbf[1][:, ts(cc, 512)], xbf_d[1, :, ts(cc, 512)])
        # residual base: out_d starts as a copy of x (DRAM->DRAM on the
        # gpsimd SWDGE queue; later accum-DMAs on the same queue add t)
        for b in range(BPC):
            for cc in range(4):
                nc.gpsimd.dma_start(out_d[b, :, ts(cc, 1024)], x_d[b, :, ts(cc, 1024)])

        # ---- phase A: conv (theta/phi replicated + g), both batches -------
        # PSUM partitions: {0,32,96}+0..7 theta, {8,40,104}+0..7 phi, 64-95 g
        thp = [
            thp_p.tile([128, N], bf16, tag=f"thp{b}", name=f"thp{b}")
            for b in range(BPC)
        ]
        pa_ctx = tc.tile_pool(name="pa_ps", bufs=2, space="PSUM")
        pa_ps = pa_ctx.__enter__()

        # HAM warm-up: keep PE busy ~7us while input DMAs land so the
        # clock-gate opens to 2.4 GHz before the real matmul stream starts
        for _ in range(16):
            wt = pa_ps.tile([128, 2048], f32, tag="pa", name="pa")
            for j in range(4):
                nc.tensor.matmul(
                    wt[:, ts(j, 512)], lhsT=w2_sb[:], rhs=xbf[0][:, 0:512],
                    start=True, stop=True,
                )

        def emit_conv(b):
            for cc in range(2):  # two 2048-wide chunks
                pa_t = pa_ps.tile([128, 2048], f32, tag="pa", name="pa")
                for j in range(4):
                    nc.tensor.matmul(
                        pa_t[:, ts(j, 512)],
                        lhsT=w2_sb[:],
                        rhs=xbf[b][:, cc * 2048 + j * 512 : cc * 2048 + (j + 1) * 512],
                        start=True,
                        stop=True,
                    )
                nc.scalar.activation(thp[b][:, ts(cc, 2048)], pa_t[:], Copy)

        # ---- phase B: 2x2 maxpool of ALL partitions at once ---------------
        pooled = [None] * BPC

        phir = [None] * BPC

        def emit_pool(b):
            pt = pgp_p.tile([128, M], bf16, tag=f"pool{b}", name=f"pool{b}")
            pooled[b] = pt
            tmp = pgp_p.tile([128, N // 2], bf16, tag="ptmp", name="ptmp")
            # stage 1: max over w-pairs (adjacent cols)
            v = thp[b][:].rearrange("p (hw t) -> p hw t", t=2)
            nc.vector.tensor_tensor(tmp[:], v[:, :, 0], v[:, :, 1], amax)
            # stage 2: max over h-pairs (stride-64 rows of the 64x32 grid)
            v2 = tmp[:].rearrange("p (h t w) -> p h t w", t=2, w=32)
            dst = pt[:].rearrange("p (h w) -> p h w", w=32)
            nc.vector.tensor_tensor(dst[:], v2[:, :, 0, :], v2[:, :, 1, :], amax)
            # move pooled-phi replicas to 32-aligned bases for LDWEIGHTS
            ph = pgp_p.tile([128, M], bf16, tag=f"phir{b}", name=f"phir{b}")
            phir[b] = ph
            for rb0 in RB:
                nc.sync.dma_start(ph[rb0 : rb0 + 8, :], pt[rb0 + 8 : rb0 + 16, :])

        # ---- G2T: fold W_o into g; fp8 DoubleRow pair layout --------------
        # pair p covers m-tiles ti=2p,2p+1; slot cols [p*160 + i*80 + c]:
        # c 0..63 = (W_o @ g)^T, c 64 = ones (denominator), c 65..79 = pad
        pg2_ctx = tc.tile_pool(name="pg2_ps", bufs=2, space="PSUM")
        pg2_ps = None
        g2t = [None] * BPC

        def emit_g2t(b):
            g2 = g2t_p.tile([128, 4 * 160], fp8, tag=f"g2t{b}", name=f"g2t{b}")
            g2t[b] = g2
            nc.gpsimd.memset(g2[:], 0.0)
            g2v = g2.rearrange("p (s c) -> p s c", c=80)
            nc.gpsimd.memset(g2v[:, :, 64:65], 1.0)
            for tq in range(2):  # 4 m-tiles per psum tile
                pg2_t = pg2_ps.tile([128, 256], f32, tag="pg2", name="pg2")
                for k in range(4):
                    ti = 4 * tq + k
                    nc.tensor.matmul(
                        pg2_t[:, ts(k, 64)],
                        lhsT=pooled[b][64:96, ts(ti, 128)],
                        rhs=wot_sb[64:96, :],
                        start=True,
                        stop=True,
                    )
                # strided copy into the 4 slot positions (stride 80)
                dst = g2[:, tq * 320 : (tq + 1) * 320].rearrange(
                    "p (s c) -> p s c", c=80
                )
                src = pg2_t[:].rearrange("p (s c) -> p s c", c=64)
                nc.scalar.activation(dst[:, :, 0:64], src[:], Copy)

        emit_conv(0)
        emit_pool(0)
        emit_conv(1)
        emit_pool(1)
        pa_ctx.__exit__(None, None, None)
        pg2_ps = pg2_ctx.__enter__()
        emit_g2t(0)
        emit_g2t(1)
        pg2_ctx.__exit__(None, None, None)

        # ---- phase C/D: scores -> exp -> av -> normalize + residual -------
        # flat pair-unit stream with software-pipelined av (skew) so the
        # in-order PE never stalls behind ScalarE/DVE exp.
        pe_ctx = tc.tile_pool(name="pe_ps", bufs=3, space="PSUM")
        pe_ps = pe_ctx.__enter__()
        pav_ctx = tc.tile_pool(name="pav_ps", bufs=2, space="PSUM")
        pav_ps = pav_ctx.__enter__()
        pvs_p = pool("pvs", 2)

        SKEW = 4
        units = []  # (nb, b, half, p)
        for nb in range(NNB):
            for b in range(BPC):
                for half in range(2):
                    for p in range(4):
                        units.append((nb, b, half, p))

        pav_tiles = {}
        ebs = {}

        def emit_scores_exp(u):
            nb, b, half, p = units[u]
            n0 = (2 * nb + half) * NCH
            pe_t = pe_ps.tile([128, 2 * NCH], f32, tag="pe", name="pe")
            for k in range(2):
                ti = 2 * p + k
                rb0 = RB[ti % 3]
                nc.tensor.matmul(
                    pe_t[:, ts(k, NCH)],
                    lhsT=phir[b][rb0 : rb0 + 8, ts(ti, 128)],
                    rhs=thp[b][rb0 : rb0 + 8, n0 : n0 + NCH],
                    start=True,
                    stop=True,
                    tile_position=(rb0, 0),
                )
            eb = e_p.tile([128, 2 * NCH], fp8, tag="e", name="e")
            ebs[u] = eb
            # exp split: ScalarE table-exp / DVE fast-exp
            if (u * 9) % 16 < 9:
                nc.scalar.activation(eb[:], pe_t[:], Exp)
            else:
                nc.vector.tensor_scalar(
                    eb[:].bitcast(i8), pe_t[:], FE_A, FE_B, amult, aadd
                )

        def emit_av(u):
            nb, b, half, p = units[u]
            key = (nb, b, half)
            if key not in pav_tiles and p == 0:
                pav_tiles[key] = pav_ps.tile([80, NCH], f32, tag="pav", name="pav")
            pav_t = pav_tiles[key]
            eb = ebs.pop(u)
            ev = eb[:].rearrange("q (i f) -> q i f", i=2)
            g2v = g2t[b][:, p * 160 : (p + 1) * 160].rearrange(
                "q (i c) -> q i c", i=2
            )
            nc.tensor.matmul(
                pav_t[:],
                lhsT=g2v[:],
                rhs=ev[:],
                start=(p == 0),
                stop=(p == 3),
                perf_mode=DR,
            )
            if p == 3:
                emit_norm(nb, b, half, pav_tiles.pop(key))

        def emit_norm(nb, b, half, pav_t):
            ch = 2 * nb + half
            ch0 = ch * NCH
            ci = 2 * (BPC * nb + b) + half  # global chunk index for scratch
            # one bf16 copy frees the PSUM accumulator; row 64 is the denom
            pavs = pvs_p.tile([65, NCH], bf16, tag="pavs", name="pavs")
            if (ch + b) % 2 == 0:
                nc.scalar.activation(pavs[:], pav_t[0:65, :], Copy)
            else:
                nc.vector.tensor_copy(pavs[:], pav_t[0:65, :])
            dsq = rec_p.tile([64, NCH // 64], bf16, tag="dsq", name="dsq")
            nc.sync.dma_start(dsq[:], pavs[64:65, :])
            rsq = rec_p.tile([64, NCH // 64], f32, tag="rsq", name="rsq")
            nc.vector.reciprocal(rsq[:], dsq[:])
            # fold gamma: rsq2 = gamma / d  (bf16; exactly 0 when gamma=0)
            rsq2 = rec_p.tile([64, NCH // 64], bf16, tag="rsq2", name="rsq2")
            nc.vector.tensor_scalar(rsq2[:], rsq[:], gcol_sb[:, 0:1], None, amult)
            # replicate gamma/d across 64 partitions via a DRAM round-trip
            # (stride-0 DMA source; no gpsimd compute -> no Q7 ucode thrash)
            nc.sync.dma_start(scr_d[ci : ci + 1, :], rsq2[:])
            rb_t = rb_p.tile([64, NCH], bf16, tag="rb", name="rb")
            nc.sync.dma_start(rb_t[:], scr_d[ci : ci + 1, :].broadcast_to([64, NCH]))
            # t = (gamma/d) * o_raw   (bf16 2x; exactly 0 when gamma=0)
            t_t = tt_p.tile([64, NCH], bf16, tag="t", name="t")
            nc.vector.tensor_tensor(t_t[:], pavs[0:64, :], rb_t[:], amult)
            # out += t via DMA accumulate (residual add on the DMA engine;
            # same SWDGE queue as the base copy -> FIFO ordering)
            nc.gpsimd.dma_start(out_d[b, :, ch0 : ch0 + NCH], t_t[:], accum_op=aadd)

        for u in range(len(units)):
            emit_scores_exp(u)
            if u >= SKEW:
                emit_av(u - SKEW)
        for u in range(len(units) - SKEW, len(units)):
            emit_av(u)
        pav_ctx.__exit__(None, None, None)
        pe_ctx.__exit__(None, None, None)

    nc.compile()
    _BUILT = nc
    return nc


def _in_maps(x, W_theta, W_phi, W_g, W_o, gamma):
    x = np.asarray(x, dtype=np.float32)
    w2 = np.zeros((128, 64), dtype=np.float32)
    for rb0 in (0, 32, 96):
        w2[rb0 : rb0 + 8] = np.asarray(W_theta)
        w2[rb0 + 8 : rb0 + 16] = np.asarray(W_phi)
    w2[64:96] = np.asarray(W_g)
    w2 = np.ascontiguousarray(w2.T).astype(ml_dtypes.bfloat16)
    wot = np.ascontiguousarray(np.asarray(W_o).T).astype(ml_dtypes.bfloat16)
    gcol = np.full((64, 1), np.float32(np.asarray(gamma)), dtype=np.float32)
    maps = []
    xbf_all = x.astype(ml_dtypes.bfloat16)
    for i in range(NCORES):
        xs = np.ascontiguousarray(x[i * BPC : (i + 1) * BPC].reshape(BPC, CH, N))
        xbfs = np.ascontiguousarray(
            xbf_all[i * BPC : (i + 1) * BPC].reshape(BPC, CH, N)
        )
        maps.append({"x": xs, "xbf": xbfs, "w2": w2, "wot": wot, "gcol": gcol})
    return maps


def run_shards(in_maps, **kw):
    nc = _build()
    from concourse.bass_utils import run_bass_kernel_spmd

    return run_bass_kernel_spmd(nc, in_maps, core_ids=list(range(NCORES)), **kw)


def kernel(x, W_theta, W_phi, W_g, W_o, gamma):
    res = run_shards(_in_maps(x, W_theta, W_phi, W_g, W_o, gamma))
    out = np.concatenate([res.results[i]["out"] for i in range(NCORES)], axis=0)
    return np.ascontiguousarray(out.reshape(B, CH, H, W).astype(np.float32))


if __name__ == "__main__":
    rng = np.random.default_rng(0)
    ins = {
        "x": rng.standard_normal((B, CH, H, W), dtype=np.float32),
        "W_theta": (rng.standard_normal((8, 64)) * 0.05).astype(np.float32),
        "W_phi": (rng.standard_normal((8, 64)) * 0.05).astype(np.float32),
        "W_g": (rng.standard_normal((32, 64)) * 0.05).astype(np.float32),
        "W_o": (rng.standard_normal((64, 32)) * 0.05).astype(np.float32),
        "gamma": np.float32(0.0),
    }
    out = kernel(**ins)
    print("out", out.shape, out.dtype, float(np.abs(out - ins["x"]).max()))


# revision 20
# speedup vs baseline: 1.3342x; 1.3342x over previous
"""SAGAN-style self-attention on 8 TRN2 NeuronCores, pure data-parallel.

Reference computation (per batch element, CH=64, H=W=64, N=4096, M=1024):
    theta = W_theta @ x          [8, N]
    phi   = pool(W_phi @ x)      [8, M]
    g     = pool(W_g @ x)        [32, M]
    E     = exp(theta^T phi)     [N, M]
    o     = W_o @ (g @ beta^T),  beta = E / rowsum(E)
    out   = gamma * o + x

Per-core strategy (2 batch elements), v2:
  - conv weight matrix [64, 128] emits theta/phi REPLICATED at partition
    bases {0, 32, 96} and g at 64-95, so score matmuls (K=8) pack 3-way
    into distinct PE row-groups via tile_position and run concurrently
    (~3x PE throughput on the output-bound scores phase).
  - 2x2 maxpool over ALL 128 partitions in one DVE pass (cost is
    free-dim-bound, so pooling the replicas costs nothing extra).
  - W_o folded into g up front (G2T pair layout for fp8 DoubleRow av
    matmul, ones-column emits the softmax denominator for free).
  - exp split across ScalarE (table Exp -> fp8) and DVE (fast-exp:
    tensor_scalar affine -> int8 bitcast as fp8e4m3; same quality class
    as fp8 quantization of exact exp).
  - normalize: denominator DMA-reshaped for 64-lane reciprocal, gamma
    folded into the reciprocal, gpsimd partition_broadcast; residual
    out = t + x on gpsimd with f32 x (bit-exact passthrough at gamma=0).
"""

import os
import sys

import numpy as np

if "/opt/trn_rl_repo" not in sys.path:
    sys.path.insert(0, "/opt/trn_rl_repo")

import ml_dtypes

B, CH, H, W = 16, 64, 64, 64
N = H * W          # 4096 queries
M = N // 4         # 1024 keys (after 2x2 pool)
NCORES = 8
BPC = B // NCORES  # 2 batch elements per core

NB = 1024          # normalize block width
NNB = N // NB      # 4 per batch element
NCH = 512          # attention n-chunk width
NCHUNK = N // NCH  # 8 chunks per batch element

# fast-exp constants: fp8e4m3 bits u = round(A*s + BEXP) reinterpreted as
# fp8 approximate exp(s).  A = 8/ln2; BEXP = 8*7 (bias) - 0.45 (mantissa
# PWL centering, Schraudolph-style).
FE_A = float(8.0 / np.log(2.0))
FE_B = 56.0 - 0.45

_BUILT = None


def _build():
    global _BUILT
    if _BUILT is not None:
        return _BUILT

    from contextlib import ExitStack

    import concourse.bass as bass
    import concourse.mybir as mybir
    import concourse.tile as tile
    from concourse import bacc

    f32 = mybir.dt.float32
    bf16 = mybir.dt.bfloat16
    fp8 = mybir.dt.float8e4
    i8 = mybir.dt.int8
    DR = mybir.MatmulPerfMode.DoubleRow
    ts = bass.ts
    Exp = mybir.ActivationFunctionType.Exp
    Copy = mybir.ActivationFunctionType.Copy
    amax = mybir.AluOpType.max
    amult = mybir.AluOpType.mult
    aadd = mybir.AluOpType.add

    nc = bacc.Bacc("TRN2", target_bir_lowering=False, debug=False)

    x_d = nc.dram_tensor("x", [BPC, 64, N], f32, kind="ExternalInput")
    xbf_d = nc.dram_tensor("xbf", [BPC, 64, N], bf16, kind="ExternalInput")
    w2_d = nc.dram_tensor("w2", [64, 128], bf16, kind="ExternalInput")
    wot_d = nc.dram_tensor("wot", [32, 64], bf16, kind="ExternalInput")
    gcol_d = nc.dram_tensor("gcol", [64, 1], f32, kind="ExternalInput")
    out_d = nc.dram_tensor("out", [BPC, 64, N], f32, kind="ExternalOutput")
    scr_d = nc.dram_tensor("scr", [16, NCH], bf16, kind="Internal")

    # theta replica bases for 3-way packed score matmuls
    RB = (0, 32, 96)

    with tile.TileContext(nc) as tc, ExitStack() as ctx:
        pool = lambda name, bufs, **kw: ctx.enter_context(
            tc.tile_pool(name=name, bufs=bufs, **kw)
        )
        const_p = pool("const", 1)
        xb_p = pool("xb", 1)
        thp_p = pool("thp", 1)
        pgp_p = pool("pg", 1)
        g2t_p = pool("g2t", 1)
        e_p = pool("esb", 8)
        rec_p = pool("rec", 2)
        rb_p = pool("rb", 2)
        tt_p = pool("tt", 2)
        ou_p = pool("ou", 3)

        # ---- load constants + inputs (xbf first: conv-critical) -----------
        xbf = []
        for b in range(BPC):
            xbf.append(xb_p.tile([64, N], bf16, tag=f"xbf{b}", name=f"xbf{b}"))
        for cc in range(2):
            nc.sync.dma_start(xbf[0][:, ts(cc, 512)], xbf_d[0, :, ts(cc, 512)])
        w2_sb = const_p.tile([64, 128], bf16, tag="w2", name="w2")
        nc.sync.dma_start(w2_sb[:], w2_d[:, :])
        wot_sb = const_p.tile([128, 64], bf16, tag="wot", name="wot")
        nc.sync.dma_start(wot_sb[64:96, :], wot_d[:, :])
        gcol_sb = const_p.tile([64, 1], f32, tag="gcol", name="gcol")
        nc.sync.dma_start(gcol_sb[:], gcol_d[:, :])
        for cc in range(2, 8):
            nc.sync.dma_start(xbf[0][:, ts(cc, 512)], xbf_d[0, :, ts(cc, 512)])
        for cc in range(8):
            nc.sync.dma_start(xbf[1][:, ts(cc, 512)], xbf_d[1, :, ts(cc, 512)])
        # residual base: out_d starts as a copy of x (DRAM->DRAM on the
        # gpsimd SWDGE queue; later accum-DMAs on the same queue add t)
        for b in range(BPC):
            for cc in range(4):
                nc.gpsimd.dma_start(out_d[b, :, ts(cc, 1024)], x_d[b, :, ts(cc, 1024)])

        # ---- phase A: conv (theta/phi replicated + g), both batches -------
        # PSUM partitions: {0,32,96}+0..7 theta, {8,40,104}+0..7 phi, 64-95 g
        thp = [
            thp_p.tile([128, N], bf16, tag=f"thp{b}", name=f"thp{b}")
            for b in range(BPC)
        ]
        pa_ctx = tc.tile_pool(name="pa_ps", bufs=2, space="PSUM")
        pa_ps = pa_ctx.__enter__()

        def emit_conv(b):
            for cc in range(2):  # two 2048-wide chunks
                pa_t = pa_ps.tile([128, 2048], f32, tag="pa", name="pa")
                for j in range(4):
                    nc.tensor.matmul(
                        pa_t[:, ts(j, 512)],
                        lhsT=w2_sb[:],
                        rhs=xbf[b][:, cc * 2048 + j * 512 : cc * 2048 + (j + 1) * 512],
                        start=True,
                        stop=True,
                    )
                nc.scalar.activation(thp[b][:, ts(cc, 2048)], pa_t[:], Copy)

        # ---- phase B: 2x2 maxpool of ALL partitions at once ---------------
        pooled = [None] * BPC

        phir = [None] * BPC

        def emit_pool(b):
            pt = pgp_p.tile([128, M], bf16, tag=f"pool{b}", name=f"pool{b}")
            pooled[b] = pt
            tmp = pgp_p.tile([128, N // 2], bf16, tag="ptmp", name="ptmp")
            # stage 1: max over w-pairs (adjacent cols)
            v = thp[b][:].rearrange("p (hw t) -> p hw t", t=2)
            nc.vector.tensor_tensor(tmp[:], v[:, :, 0], v[:, :, 1], amax)
            # stage 2: max over h-pairs (stride-64 rows of the 64x32 grid)
            v2 = tmp[:].rearrange("p (h t w) -> p h t w", t=2, w=32)
            dst = pt[:].rearrange("p (h w) -> p h w", w=32)
            nc.vector.tensor_tensor(dst[:], v2[:, :, 0, :], v2[:, :, 1, :], amax)
            # move pooled-phi replicas to 32-aligned bases for LDWEIGHTS
            ph = pgp_p.tile([128, M], bf16, tag=f"phir{b}", name=f"phir{b}")
            phir[b] = ph
            for rb0 in RB:
                nc.sync.dma_start(ph[rb0 : rb0 + 8, :], pt[rb0 + 8 : rb0 + 16, :])

        # ---- G2T: fold W_o into g; fp8 DoubleRow pair layout --------------
        # pair p covers m-tiles ti=2p,2p+1; slot cols [p*160 + i*80 + c]:
        # c 0..63 = (W_o @ g)^T, c 64 = ones (denominator), c 65..79 = pad
        pg2_ctx = tc.tile_pool(name="pg2_ps", bufs=2, space="PSUM")
        pg2_ps = None
        g2t = [None] * BPC

        def emit_g2t(b):
            g2 = g2t_p.tile([128, 4 * 160], fp8, tag=f"g2t{b}", name=f"g2t{b}")
            g2t[b] = g2
            nc.gpsimd.memset(g2[:], 0.0)
            g2v = g2.rearrange("p (s c) -> p s c", c=80)
            nc.gpsimd.memset(g2v[:, :, 64:65], 1.0)
            for tq in range(2):  # 4 m-tiles per psum tile
                pg2_t = pg2_ps.tile([128, 256], f32, tag="pg2", name="pg2")
                for k in range(4):
                    ti = 4 * tq + k
                    nc.tensor.matmul(
                        pg2_t[:, ts(k, 64)],
                        lhsT=pooled[b][64:96, ts(ti, 128)],
                        rhs=wot_sb[64:96, :],
                        start=True,
                        stop=True,
                    )
                # strided copy into the 4 slot positions (stride 80)
                dst = g2[:, tq * 320 : (tq + 1) * 320].rearrange(
                    "p (s c) -> p s c", c=80
                )
                src = pg2_t[:].rearrange("p (s c) -> p s c", c=64)
                nc.scalar.activation(dst[:, :, 0:64], src[:], Copy)

        emit_conv(0)
        emit_pool(0)
        emit_conv(1)
        emit_pool(1)
        pa_ctx.__exit__(None, None, None)
        pg2_ps = pg2_ctx.__enter__()
        emit_g2t(0)
        emit_g2t(1)
        pg2_ctx.__exit__(None, None, None)

        # ---- phase C/D: scores -> exp -> av -> normalize + residual -------
        # flat pair-unit stream with software-pipelined av (skew) so the
        # in-order PE never stalls behind ScalarE/DVE exp.
        pe_ctx = tc.tile_pool(name="pe_ps", bufs=3, space="PSUM")
        pe_ps = pe_ctx.__enter__()
        pav_ctx = tc.tile_pool(name="pav_ps", bufs=2, space="PSUM")
        pav_ps = pav_ctx.__enter__()
        pvs_p = pool("pvs", 2)

        SKEW = 4
        units = []  # (nb, b, half, p)
        for nb in range(NNB):
            for b in range(BPC):
                for half in range(2):
                    for p in range(4):
                        units.append((nb, b, half, p))

        pav_tiles = {}
        ebs = {}

        def emit_scores_exp(u):
            nb, b, half, p = units[u]
            n0 = (2 * nb + half) * NCH
            pe_t = pe_ps.tile([128, 2 * NCH], f32, tag="pe", name="pe")
            for k in range(2):
                ti = 2 * p + k
                rb0 = RB[ti % 3]
                nc.tensor.matmul(
                    pe_t[:, ts(k, NCH)],
                    lhsT=phir[b][rb0 : rb0 + 8, ts(ti, 128)],
                    rhs=thp[b][rb0 : rb0 + 8, n0 : n0 + NCH],
                    start=True,
                    stop=True,
                    tile_position=(rb0, 0),
                )
            eb = e_p.tile([128, 2 * NCH], fp8, tag="e", name="e")
            ebs[u] = eb
            # exp split: ScalarE table-exp / DVE fast-exp
            if (u * 9) % 16 < 9:
                nc.scalar.activation(eb[:], pe_t[:], Exp)
            else:
                nc.vector.tensor_scalar(
                    eb[:].bitcast(i8), pe_t[:], FE_A, FE_B, amult, aadd
                )

        def emit_av(u):
            nb, b, half, p = units[u]
            key = (nb, b, half)
            if key not in pav_tiles and p == 0:
                pav_tiles[key] = pav_ps.tile([80, NCH], f32, tag="pav", name="pav")
            pav_t = pav_tiles[key]
            eb = ebs.pop(u)
            ev = eb[:].rearrange("q (i f) -> q i f", i=2)
            g2v = g2t[b][:, p * 160 : (p + 1) * 160].rearrange(
                "q (i c) -> q i c", i=2
            )
            nc.tensor.matmul(
                pav_t[:],
                lhsT=g2v[:],
                rhs=ev[:],
                start=(p == 0),
                stop=(p == 3),
                perf_mode=DR,
            )
            if p == 3:
                emit_norm(nb, b, half, pav_tiles.pop(key))

        def emit_norm(nb, b, half, pav_t):
            ch = 2 * nb + half
            ch0 = ch * NCH
            ci = 2 * (BPC * nb + b) + half  # global chunk index for scratch
            # one bf16 copy frees the PSUM accumulator; row 64 is the denom
            pavs = pvs_p.tile([65, NCH], bf16, tag="pavs", name="pavs")
            if (ch + b) % 2 == 0:
                nc.scalar.activation(pavs[:], pav_t[0:65, :], Copy)
            else:
                nc.vector.tensor_copy(pavs[:], pav_t[0:65, :])
            dsq = rec_p.tile([64, NCH // 64], bf16, tag="dsq", name="dsq")
            nc.sync.dma_start(dsq[:], pavs[64:65, :])
            rsq = rec_p.tile([64, NCH // 64], f32, tag="rsq", name="rsq")
            nc.vector.reciprocal(rsq[:], dsq[:])
            # fold gamma: rsq2 = gamma / d  (bf16; exactly 0 when gamma=0)
            rsq2 = rec_p.tile([64, NCH // 64], bf16, tag="rsq2", name="rsq2")
            nc.vector.tensor_scalar(rsq2[:], rsq[:], gcol_sb[:, 0:1], None, amult)
            # replicate gamma/d across 64 partitions via a DRAM round-trip
            # (stride-0 DMA source; no gpsimd compute -> no Q7 ucode thrash)
            nc.sync.dma_start(scr_d[ci : ci + 1, :], rsq2[:])
            rb_t = rb_p.tile([64, NCH], bf16, tag="rb", name="rb")
            nc.sync.dma_start(rb_t[:], scr_d[ci : ci + 1, :].broadcast_to([64, NCH]))
            # t = (gamma/d) * o_raw   (bf16 2x; exactly 0 when gamma=0)
            t_t = tt_p.tile([64, NCH], bf16, tag="t", name="t")
            nc.vector.tensor_tensor(t_t[:], pavs[0:64, :], rb_t[:], amult)
            # out += t via DMA accumulate (residual add on the DMA engine;
            # same SWDGE queue as the base copy -> FIFO ordering)
            nc.gpsimd.dma_start(out_d[b, :, ch0 : ch0 + NCH], t_t[:], accum_op=aadd)

        for u in range(len(units)):
            emit_scores_exp(u)
            if u >= SKEW:
                emit_av(u - SKEW)
        for u in range(len(units) - SKEW, len(units)):
            emit_av(u)
        pav_ctx.__exit__(None, None, None)
        pe_ctx.__exit__(None, None, None)

    nc.compile()
    _BUILT = nc
    return nc


def _in_maps(x, W_theta, W_phi, W_g, W_o, gamma):
    x = np.asarray(x, dtype=np.float32)
    w2 = np.zeros((128, 64), dtype=np.float32)
    for rb0 in (0, 32, 96):
        w2[rb0 : rb0 + 8] = np.asarray(W_theta)
        w2[rb0 + 8 : rb0 + 16] = np.asarray(W_phi)
    w2[64:96] = np.asarray(W_g)
    w2 = np.ascontiguousarray(w2.T).astype(ml_dtypes.bfloat16)
    wot = np.ascontiguousarray(np.asarray(W_o).T).astype(ml_dtypes.bfloat16)
    gcol = np.full((64, 1), np.float32(np.asarray(gamma)), dtype=np.float32)
    maps = []
    xbf_all = x.astype(ml_dtypes.bfloat16)
    for i in range(NCORES):
        xs = np.ascontiguousarray(x[i * BPC : (i + 1) * BPC].reshape(BPC, CH, N))
        xbfs = np.ascontiguousarray(
            xbf_all[i * BPC : (i + 1) * BPC].reshape(BPC, CH, N)
        )
        maps.append({"x": xs, "xbf": xbfs, "w2": w2, "wot": wot, "gcol": gcol})
    return maps


def run_shards(in_maps, **kw):
    nc = _build()
    from concourse.bass_utils import run_bass_kernel_spmd

    return run_bass_kernel_spmd(nc, in_maps, core_ids=list(range(NCORES)), **kw)


def kernel(x, W_theta, W_phi, W_g, W_o, gamma):
    res = run_shards(_in_maps(x, W_theta, W_phi, W_g, W_o, gamma))
    out = np.concatenate([res.results[i]["out"] for i in range(NCORES)], axis=0)
    return np.ascontiguousarray(out.reshape(B, CH, H, W).astype(np.float32))


if __name__ == "__main__":
    rng = np.random.default_rng(0)
    ins = {
        "x": rng.standard_normal((B, CH, H, W), dtype=np.float32),
        "W_theta": (rng.standard_normal((8, 64)) * 0.05).astype(np.float32),
        "W_phi": (rng.standard_normal((8, 64)) * 0.05).astype(np.float32),
        "W_g": (rng.standard_normal((32, 64)) * 0.05).astype(np.float32),
        "W_o": (rng.standard_normal((64, 32)) * 0.05).astype(np.float32),
        "gamma": np.float32(0.0),
    }
    out = kernel(**ins)
    print("out", out.shape, out.dtype, float(np.abs(out - ins["x"]).max()))
